# revision 19
# baseline (speedup 1.0000x reference)
"""Trainium2 Bass kernel for GQA attention (RoPE + causal) with output projection.

Strategy: tensor-parallel over heads across 8 NeuronCores. Core c computes
q-heads {2c, 2c+1} and kv-head c//2, projects with its weight slices, runs
causal flash-style attention in scores-transposed layout, applies its slice
of wo, and returns a full-shape partial output (bf16). The host sums the 8
partials (the all-reduce of the TP layout).

v2 changes vs baseline:
- all matmul operands bf16 (host-converted): enables Fast Weight Load,
  halves DMA + SBUF traffic; PSUM accumulation stays fp32.
- causal band tiles compute only live columns (skips ~15% of P2 work).
- softmax denominator: reciprocal_approx_fast (5x faster than reciprocal).
- rsum accumulation moved to the (otherwise idle) GpSimd engine where it is
  SBUF-only; PSUM evacuations split between Act and DVE.
- 8-bank PSUM budget: shared 4-slot rotating pool (P1 accs / rope swaps /
  transposes / P3 tiles / softmax sums) + 2 score bufs + 2 PV accumulators.
- one DMA per x chunk; bf16 partial outputs.
"""

import math
from contextlib import ExitStack
from dataclasses import dataclass

import numpy as np
import ml_dtypes

import concourse.bass as bass
import concourse.tile as tile
from concourse import bacc, mybir
from concourse.bass_utils import run_bass_kernel_spmd

F32 = mybir.dt.float32
F32R = mybir.dt.float32r
BF16 = mybir.dt.bfloat16
AF = mybir.ActivationFunctionType
MUL = mybir.AluOpType.mult
ADD = mybir.AluOpType.add

BF = ml_dtypes.bfloat16


@dataclass(frozen=True)
class Cfg:
    B: int = 4          # batch
    S: int = 2048       # sequence length
    D: int = 2048       # model dim
    HQC: int = 2        # q-heads per core
    HD: int = 128       # head dim (must be 128)
    QCH: int = 512      # q-chunk (matmul moving free dim)

    @property
    def DT(self):
        return self.D // 128   # d-tiles

    @property
    def KT(self):
        return self.S // 128   # k-tiles / s-tiles / q-tiles

    @property
    def NQC(self):
        return self.S // self.QCH  # q-chunks

    @property
    def RB(self):
        return self.QCH // 128     # band tiles per q-chunk

    @property
    def NDC(self):
        return self.D // self.QCH  # dout chunks


def r(ap):
    """View an fp32 AP as float32r for full-rate PE matmuls."""
    return ap.bitcast(F32R)


def build_program(cfg: Cfg):
    """Build + compile the single-core Bass program (same program on every core)."""
    c = cfg
    assert c.HD == 128
    nc = bacc.Bacc("TRN2", target_bir_lowering=False, debug=False)

    xt_d = nc.dram_tensor("xt", [c.B, c.D, c.S], BF16, kind="ExternalInput")
    wqt_d = nc.dram_tensor("wqt", [c.D, c.HQC * c.HD], BF16, kind="ExternalInput")
    wkt_d = nc.dram_tensor("wkt", [c.D, c.HD], BF16, kind="ExternalInput")
    wvt_d = nc.dram_tensor("wvt", [c.D, c.HD], BF16, kind="ExternalInput")
    wot_d = nc.dram_tensor("wot", [c.HQC * c.HD, c.D], BF16, kind="ExternalInput")
    ra_d = nc.dram_tensor("ra", [c.HD, c.S], BF16, kind="ExternalInput")
    rb_d = nc.dram_tensor("rb", [c.HD, c.S], BF16, kind="ExternalInput")
    cm_d = nc.dram_tensor("cm", [128, 128], BF16, kind="ExternalInput")
    id_d = nc.dram_tensor("id", [128, 128], BF16, kind="ExternalInput")
    pm_d = nc.dram_tensor("pm", [128, 128], BF16, kind="ExternalInput")
    out_d = nc.dram_tensor("partial", [c.B, c.S, c.D], BF16, kind="ExternalOutput")

    scale = 1.0 / math.sqrt(c.HD)

    with tile.TileContext(nc) as tc, ExitStack() as ctx:
        const = ctx.enter_context(tc.tile_pool(name="const", bufs=1))
        xp = ctx.enter_context(tc.tile_pool(name="xp", bufs=1))
        qk = ctx.enter_context(tc.tile_pool(name="qk", bufs=1))
        ptp = ctx.enter_context(tc.tile_pool(name="ptp", bufs=1))
        rsp = ctx.enter_context(tc.tile_pool(name="rsp", bufs=1))
        atp = ctx.enter_context(tc.tile_pool(name="atp", bufs=1))
        orp = ctx.enter_context(tc.tile_pool(name="orp", bufs=1))
        # PSUM: mix (2) + stp (2) + otp (2) + o3 (2) = 8 banks
        mixp = ctx.enter_context(
            tc.tile_pool(name="mixp", bufs=2, space=bass.MemorySpace.PSUM)
        )
        stpp = ctx.enter_context(
            tc.tile_pool(name="stpp", bufs=2, space=bass.MemorySpace.PSUM)
        )
        otpp = ctx.enter_context(
            tc.tile_pool(name="otpp", bufs=2, space=bass.MemorySpace.PSUM)
        )
        o3p = ctx.enter_context(
            tc.tile_pool(name="o3p", bufs=2, space=bass.MemorySpace.PSUM)
        )

        # ---- resident constants ----
        wq_sb = const.tile([128, c.DT, c.HQC * c.HD], BF16, name="wq_sb")
        nc.sync.dma_start(wq_sb[:], wqt_d.rearrange("(t p) h -> p t h", p=128))
        wk_sb = const.tile([128, c.DT, c.HD], BF16, name="wk_sb")
        nc.sync.dma_start(wk_sb[:], wkt_d.rearrange("(t p) h -> p t h", p=128))
        wv_sb = const.tile([128, c.DT, c.HD], BF16, name="wv_sb")
        nc.sync.dma_start(wv_sb[:], wvt_d.rearrange("(t p) h -> p t h", p=128))
        wo_sb = const.tile([128, c.HQC, c.D], BF16, name="wo_sb")
        nc.sync.dma_start(wo_sb[:], wot_d.rearrange("(h p) d -> p h d", p=128))
        ra_sb = const.tile([128, c.S], BF16, name="ra_sb")
        nc.sync.dma_start(ra_sb[:], ra_d[:])
        rb_sb = const.tile([128, c.S], BF16, name="rb_sb")
        nc.sync.dma_start(rb_sb[:], rb_d[:])
        cm_sb = const.tile([128, 128], BF16, name="cm_sb")
        nc.sync.dma_start(cm_sb[:], cm_d[:])
        id_sb = const.tile([128, 128], BF16, name="id_sb")
        nc.sync.dma_start(id_sb[:], id_d[:])
        pm_sb = const.tile([128, 128], BF16, name="pm_sb")
        nc.sync.dma_start(pm_sb[:], pm_d[:])
        ones_b = const.tile([128, 1], BF16, name="ones_b")
        nc.vector.memset(ones_b[:], 1.0)

        # ---- Phase 3 as a lazy op stream -----------------------------------
        # The output projection of batch b is emitted interleaved into batch
        # b+1's P1/P2 instruction stream: its matmuls fill the PE's wait slots
        # while the Act exp paces the attention inner loop.
        def p3_gen(bb, bats):
            for qt_i in range(c.KT):
                orow = orp.tile([128, c.D], BF16, name="orow", tag="orow",
                                bufs=2)
                for dc in range(c.NDC):
                    dsl = slice(dc * c.QCH, (dc + 1) * c.QCH)
                    o3 = o3p.tile([128, c.QCH], F32, name="o3", tag="o3")
                    for h in range(c.HQC):
                        nc.tensor.matmul(
                            o3[:],
                            bats[h][:, qt_i * 128:(qt_i + 1) * 128],
                            wo_sb[:, h, dsl],
                            start=(h == 0), stop=(h == c.HQC - 1),
                        )
                        yield True
                    # evacuate on DVE only: the Act queue paces P2's exps
                    nc.vector.tensor_copy(orow[:, dsl], o3[:])
                    yield False
                nc.sync.dma_start(
                    out_d[bb, qt_i * 128:(qt_i + 1) * 128, :], orow[:]
                )
                yield False

        p3s = iter(())

        def pump(mm_budget):
            # emit deferred P3 ops until mm_budget matmuls have been issued
            while mm_budget > 0:
                is_mm = next(p3s, None)
                if is_mm is None:
                    return
                if is_mm:
                    mm_budget -= 1

        for b in range(c.B):
            # ============ Phase 1: Q^T / K^T / V^T projections ============
            qts = [
                qk.tile([128, c.S], BF16, name=f"qt{h}", tag=f"qt{h}", bufs=2)
                for h in range(c.HQC)
            ]
            kt_sb = qk.tile([128, c.S], BF16, name="kt_sb", tag="kt_sb", bufs=2)
            vt_sb = qk.tile([128, c.S], BF16, name="vt_sb", tag="vt_sb", bufs=2)
            vn = qk.tile([128, c.KT, c.HD], BF16, name="vn", tag="vn", bufs=2)

            for sc in range(c.NQC):
                sl = slice(sc * c.QCH, (sc + 1) * c.QCH)
                xt = xp.tile([128, c.DT, c.QCH], BF16, name="xt_t", tag="xt_t",
                             bufs=2)
                nc.sync.dma_start(
                    xt[:], xt_d[b].rearrange("(t p) s -> p t s", p=128)[:, :, sl]
                )
                # P1 accumulators borrow the P2 pools' banks (phases never
                # hold them concurrently): 2 from stpp, 1 from otpp, 1 mixp.
                acc = [
                    stpp.tile([128, c.QCH], F32, name="acc0", tag="stp"),
                    stpp.tile([128, c.QCH], F32, name="acc1", tag="stp"),
                    otpp.tile([128, c.QCH], F32, name="acc2", tag="ot"),
                    mixp.tile([128, c.QCH], F32, name="acc3", tag="mix"),
                ]
                for dt in range(c.DT):
                    st, sp = dt == 0, dt == c.DT - 1
                    pump(1)  # previous batch's P3 matmuls as PE fillers
                    for h in range(c.HQC):
                        nc.tensor.matmul(
                            acc[h][:],
                            wq_sb[:, dt, h * c.HD:(h + 1) * c.HD],
                            xt[:, dt, :], start=st, stop=sp,
                        )
                    nc.tensor.matmul(
                        acc[c.HQC][:], wk_sb[:, dt, :], xt[:, dt, :],
                        start=st, stop=sp,
                    )
                    nc.tensor.matmul(
                        acc[c.HQC + 1][:], wv_sb[:, dt, :], xt[:, dt, :],
                        start=st, stop=sp,
                    )
                # evacuate PSUM -> bf16 SBUF (Act + DVE split)
                nc.scalar.copy(qts[0][:, sl], acc[0][:])
                nc.vector.tensor_copy(qts[1][:, sl], acc[1][:])
                nc.scalar.copy(kt_sb[:, sl], acc[c.HQC][:])
                nc.vector.tensor_copy(vt_sb[:, sl], acc[c.HQC + 1][:])

                # rope on this chunk of q0/q1/k (in place); the cross-partition
                # pair swap runs on the PE via a permutation matmul.
                for t in (qts[0], qts[1], kt_sb):
                    swp = mixp.tile([128, c.QCH], F32, name="swp", tag="mix")
                    nc.tensor.matmul(swp[:], pm_sb[:], t[:, sl])
                    tmp = ptp.tile([128, c.QCH], BF16, name="rtmp", tag="rtmp",
                                   bufs=2)
                    nc.vector.tensor_tensor(tmp[:], swp[:], rb_sb[:, sl], MUL)
                    nc.vector.tensor_tensor(t[:, sl], t[:, sl], ra_sb[:, sl], MUL)
                    nc.vector.tensor_tensor(t[:, sl], t[:, sl], tmp[:], ADD)

                # V^T -> V natural via PE transposes
                for i in range(c.RB):
                    st_i = sc * c.RB + i
                    tp = mixp.tile([128, 128], BF16, name="tp", tag="mix")
                    nc.tensor.transpose(
                        tp[:], vt_sb[:, st_i * 128:(st_i + 1) * 128], id_sb[:]
                    )
                    nc.scalar.copy(vn[:, st_i, :], tp[:])

            # ============ Phase 2: causal attention, S^T layout ============
            ats = [
                atp.tile([128, c.S], BF16, name=f"at{h}", tag=f"at{h}", bufs=2)
                for h in range(c.HQC)
            ]
            for h in range(c.HQC):
                qt = qts[h]
                for qc in range(c.NQC):
                    qs = qc * c.QCH
                    nkt = c.RB * (qc + 1)
                    ot = otpp.tile([128, c.QCH], F32, name="ot", tag="ot")
                    # softmax denominator accumulator: ones^T @ pt summed over
                    # the kt loop on the PE (bf16 matmul, fp32 PSUM accum)
                    zps = mixp.tile([1, c.QCH], F32, name="zps", tag="mix")
                    # kt loop software-pipelined by one iteration: the PE
                    # issues score(kt+1) before the exp(kt)-dependent colsum
                    # and PV matmuls so it never idles waiting on the Act exp.
                    def consume(pt, lv, kt):
                        nc.tensor.matmul(
                            zps[:, lv:], ones_b[:], pt[:, lv:],
                            start=(kt == 0), stop=(kt == nkt - 1),
                            skip_group_check=True,
                        )
                        nc.tensor.matmul(
                            ot[:, lv:], vn[:, kt, :], pt[:, lv:],
                            start=(kt == 0), stop=(kt == nkt - 1),
                            skip_group_check=True,
                        )

                    pending = None
                    for kt in range(nkt):
                        ridx = kt - (nkt - c.RB)
                        lv = 128 * ridx if ridx > 0 else 0  # first live column
                        stp = stpp.tile([128, c.QCH], F32, name="stp", tag="stp")
                        nc.tensor.matmul(
                            stp[:, lv:],
                            kt_sb[:, kt * 128:(kt + 1) * 128],
                            qt[:, qs + lv:qs + c.QCH],
                        )
                        pump(1)  # PE filler while the Act exp runs
                        if pending is not None:
                            consume(*pending)
                        pt = ptp.tile([128, c.QCH], BF16, name="pt", tag="pt",
                                      bufs=6)
                        nc.scalar.activation(
                            pt[:, lv:], stp[:, lv:], AF.Exp, scale=scale
                        )
                        if ridx >= 0:  # mask the diagonal 128-wide block
                            nc.vector.tensor_tensor(
                                pt[:, lv:lv + 128], pt[:, lv:lv + 128],
                                cm_sb[:], MUL,
                            )
                        pending = (pt, lv, kt)
                    consume(*pending)
                    # reciprocal + broadcast of the denominator row
                    zr = rsp.tile([1, c.QCH], F32, name="zr", tag="zr", bufs=2)
                    nc.vector.reciprocal_approx_fast(zr[:], zps[:])
                    zb = rsp.tile([128, c.QCH], F32, name="zb", tag="zb",
                                  bufs=2)
                    nc.gpsimd.partition_broadcast(zb[:], zr[:])
                    nc.vector.tensor_tensor(ats[h][:, qs:qs + c.QCH], ot[:],
                                            zb[:], MUL)

            # drain any leftovers of batch b-1's P3, then queue batch b's P3
            # for interleaved emission during batch b+1
            pump(10 ** 9)
            p3s = p3_gen(b, ats)

        pump(10 ** 9)  # final batch's P3

    nc.compile()
    nc.finalize()
    return nc


# ---------------------------------------------------------------------------
# Host-side sharding / gathering
# ---------------------------------------------------------------------------

def host_prep(x, freq_cis, wq, wk, wv, wo, n_cores, cfg: Cfg):
    """Build per-core input maps (numpy only)."""
    c = cfg
    B, S, D, HD, HQC = c.B, c.S, c.D, c.HD, c.HQC
    H = wq.shape[0] // HD
    HKV = wk.shape[0] // HD
    rep = H // HKV

    x = np.asarray(x, np.float32)
    freq_cis = np.asarray(freq_cis, np.float32)
    wq = np.asarray(wq, np.float32)
    wk = np.asarray(wk, np.float32)
    wv = np.asarray(wv, np.float32)
    wo = np.asarray(wo, np.float32)

    xT = np.ascontiguousarray(x.transpose(0, 2, 1)).astype(BF)  # [B, D, S]

    # rope tables, interleaved layout: out[p] = ra[p]*t[p] + rb[p]*t[partner(p)]
    # with partner(2p) = 2p+1, partner(2p+1) = 2p
    a = freq_cis[:, :, 0, 0].T  # [HD/2, S]
    bb = freq_cis[:, :, 0, 1].T
    cc = freq_cis[:, :, 1, 0].T
    dd = freq_cis[:, :, 1, 1].T
    S_ = freq_cis.shape[0]
    ra = np.empty((HD, S_), np.float32)
    rb = np.empty((HD, S_), np.float32)
    ra[0::2], ra[1::2] = a, dd
    rb[0::2], rb[1::2] = bb, cc

    # pair-swap permutation matrix (symmetric involution)
    pm = np.zeros((HD, HD), np.float32)
    idx = np.arange(HD)
    pm[idx, idx ^ 1] = 1.0

    # causal mask for the diagonal 128x128 block: keep k <= q
    ks = np.arange(128)[:, None]
    qs = np.arange(128)[None, :]
    cm = (ks <= qs).astype(np.float32)
    ident = np.eye(128, dtype=np.float32)

    in_maps = []
    for core in range(n_cores):
        h0 = core * HQC
        kvh = h0 // rep
        wq_c = wq[h0 * HD:(h0 + HQC) * HD]
        wk_c = wk[kvh * HD:(kvh + 1) * HD]
        wv_c = wv[kvh * HD:(kvh + 1) * HD]
        wo_c = wo[:, h0 * HD:(h0 + HQC) * HD]
        in_maps.append({
            "xt": xT,
            "wqt": np.ascontiguousarray(wq_c.T).astype(BF),
            "wkt": np.ascontiguousarray(wk_c.T).astype(BF),
            "wvt": np.ascontiguousarray(wv_c.T).astype(BF),
            "wot": np.ascontiguousarray(wo_c.T).astype(BF),
            "ra": ra.astype(BF),
            "rb": rb.astype(BF),
            "cm": cm.astype(BF),
            "id": ident.astype(BF),
            "pm": pm.astype(BF),
        })
    return in_maps


def run(inputs: dict, n_cores: int = 8, cfg: Cfg = Cfg(), trace: bool = False):
    in_maps = host_prep(
        inputs["x"], inputs["freq_cis"], inputs["wq"], inputs["wk"],
        inputs["wv"], inputs["wo"], n_cores, cfg,
    )
    nc = build_program(cfg)
    res = run_bass_kernel_spmd(nc, in_maps, list(range(n_cores)), trace=trace)
    out = res.results[0]["partial"].astype(np.float32)
    for core in range(1, n_cores):
        out += res.results[core]["partial"].astype(np.float32)
    return out, res


def kernel(**inputs) -> np.ndarray:
    out, _ = run(inputs, n_cores=8, cfg=Cfg())
    return out


# revision 24
# speedup vs baseline: 1.1690x; 1.1690x over previous
"""Trainium2 Bass kernel for GQA attention (RoPE + causal) with output projection.

Strategy: tensor-parallel over heads across 8 NeuronCores. Core c computes
q-heads {2c, 2c+1} and kv-head c//2, projects with its weight slices, runs
causal flash-style attention in scores-transposed layout, applies its slice
of wo, and returns a full-shape partial output (bf16). The host sums the 8
partials (the all-reduce of the TP layout).

v2 changes vs baseline:
- all matmul operands bf16 (host-converted): enables Fast Weight Load,
  halves DMA + SBUF traffic; PSUM accumulation stays fp32.
- causal band tiles compute only live columns (skips ~15% of P2 work).
- softmax denominator: reciprocal_approx_fast (5x faster than reciprocal).
- rsum accumulation moved to the (otherwise idle) GpSimd engine where it is
  SBUF-only; PSUM evacuations split between Act and DVE.
- 8-bank PSUM budget: shared 4-slot rotating pool (P1 accs / rope swaps /
  transposes / P3 tiles / softmax sums) + 2 score bufs + 2 PV accumulators.
- one DMA per x chunk; bf16 partial outputs.
"""

import math
from contextlib import ExitStack
from dataclasses import dataclass

import numpy as np
import ml_dtypes

import concourse.bass as bass
import concourse.tile as tile
from concourse import bacc, mybir
from concourse.bass_utils import run_bass_kernel_spmd

F32 = mybir.dt.float32
F32R = mybir.dt.float32r
BF16 = mybir.dt.bfloat16
AF = mybir.ActivationFunctionType
MUL = mybir.AluOpType.mult
ADD = mybir.AluOpType.add

BF = ml_dtypes.bfloat16


@dataclass(frozen=True)
class Cfg:
    B: int = 4          # batch
    S: int = 2048       # sequence length
    D: int = 2048       # model dim
    HQC: int = 2        # q-heads per core
    HD: int = 128       # head dim (must be 128)
    QCH: int = 512      # q-chunk (matmul moving free dim)

    @property
    def DT(self):
        return self.D // 128   # d-tiles

    @property
    def KT(self):
        return self.S // 128   # k-tiles / s-tiles / q-tiles

    @property
    def NQC(self):
        return self.S // self.QCH  # q-chunks

    @property
    def RB(self):
        return self.QCH // 128     # band tiles per q-chunk

    @property
    def NDC(self):
        return self.D // self.QCH  # dout chunks


def r(ap):
    """View an fp32 AP as float32r for full-rate PE matmuls."""
    return ap.bitcast(F32R)


def build_program(cfg: Cfg):
    """Build + compile the single-core Bass program (same program on every core)."""
    c = cfg
    assert c.HD == 128
    nc = bacc.Bacc("TRN2", target_bir_lowering=False, debug=False)

    xt_d = nc.dram_tensor("xt", [c.B, c.D, c.S], BF16, kind="ExternalInput")
    wqt_d = nc.dram_tensor("wqt", [c.D, c.HQC * c.HD], BF16, kind="ExternalInput")
    wkt_d = nc.dram_tensor("wkt", [c.D, c.HD], BF16, kind="ExternalInput")
    wvt_d = nc.dram_tensor("wvt", [c.D, c.HD], BF16, kind="ExternalInput")
    wot_d = nc.dram_tensor("wot", [c.HQC * c.HD, c.D], BF16, kind="ExternalInput")
    ra_d = nc.dram_tensor("ra", [c.HD, c.S], BF16, kind="ExternalInput")
    rb_d = nc.dram_tensor("rb", [c.HD, c.S], BF16, kind="ExternalInput")
    cm_d = nc.dram_tensor("cm", [128, 128], BF16, kind="ExternalInput")
    id_d = nc.dram_tensor("id", [128, 128], BF16, kind="ExternalInput")
    pm_d = nc.dram_tensor("pm", [128, 128], BF16, kind="ExternalInput")
    out_d = nc.dram_tensor("partial", [c.B, c.S, c.D], BF16, kind="ExternalOutput")

    scale = 1.0 / math.sqrt(c.HD)

    with tile.TileContext(nc) as tc, ExitStack() as ctx:
        const = ctx.enter_context(tc.tile_pool(name="const", bufs=1))
        xp = ctx.enter_context(tc.tile_pool(name="xp", bufs=1))
        qk = ctx.enter_context(tc.tile_pool(name="qk", bufs=1))
        ptp = ctx.enter_context(tc.tile_pool(name="ptp", bufs=1))
        rsp = ctx.enter_context(tc.tile_pool(name="rsp", bufs=1))
        atp = ctx.enter_context(tc.tile_pool(name="atp", bufs=1))
        orp = ctx.enter_context(tc.tile_pool(name="orp", bufs=1))
        # PSUM: mix (4 x 1 bank rotation) + stp (2) + otp (2) = 8 banks
        mixp = ctx.enter_context(
            tc.tile_pool(name="mixp", bufs=4, space=bass.MemorySpace.PSUM)
        )
        stpp = ctx.enter_context(
            tc.tile_pool(name="stpp", bufs=2, space=bass.MemorySpace.PSUM)
        )
        otpp = ctx.enter_context(
            tc.tile_pool(name="otpp", bufs=2, space=bass.MemorySpace.PSUM)
        )

        # ---- resident constants ----
        wq_sb = const.tile([128, c.DT, c.HQC * c.HD], BF16, name="wq_sb")
        nc.sync.dma_start(wq_sb[:], wqt_d.rearrange("(t p) h -> p t h", p=128))
        wk_sb = const.tile([128, c.DT, c.HD], BF16, name="wk_sb")
        nc.sync.dma_start(wk_sb[:], wkt_d.rearrange("(t p) h -> p t h", p=128))
        wv_sb = const.tile([128, c.DT, c.HD], BF16, name="wv_sb")
        nc.sync.dma_start(wv_sb[:], wvt_d.rearrange("(t p) h -> p t h", p=128))
        wo_sb = const.tile([128, c.HQC, c.D], BF16, name="wo_sb")
        nc.sync.dma_start(wo_sb[:], wot_d.rearrange("(h p) d -> p h d", p=128))
        ra_sb = const.tile([128, c.S], BF16, name="ra_sb")
        nc.sync.dma_start(ra_sb[:], ra_d[:])
        rb_sb = const.tile([128, c.S], BF16, name="rb_sb")
        nc.sync.dma_start(rb_sb[:], rb_d[:])
        cm_sb = const.tile([128, 128], BF16, name="cm_sb")
        nc.sync.dma_start(cm_sb[:], cm_d[:])
        id_sb = const.tile([128, 128], BF16, name="id_sb")
        nc.sync.dma_start(id_sb[:], id_d[:])
        pm_sb = const.tile([128, 128], BF16, name="pm_sb")
        nc.sync.dma_start(pm_sb[:], pm_d[:])
        ones_b = const.tile([128, 1], BF16, name="ones_b")
        nc.vector.memset(ones_b[:], 1.0)

        for b in range(c.B):
            # ============ Phase 1: Q^T / K^T / V^T projections ============
            qts = [
                qk.tile([128, c.S], BF16, name=f"qt{h}", tag=f"qt{h}", bufs=2)
                for h in range(c.HQC)
            ]
            kt_sb = qk.tile([128, c.S], BF16, name="kt_sb", tag="kt_sb", bufs=2)
            vt_sb = qk.tile([128, c.S], BF16, name="vt_sb", tag="vt_sb", bufs=2)
            vn = qk.tile([128, c.KT, c.HD], BF16, name="vn", tag="vn", bufs=2)

            for sc in range(c.NQC):
                sl = slice(sc * c.QCH, (sc + 1) * c.QCH)
                xt = xp.tile([128, c.DT, c.QCH], BF16, name="xt_t", tag="xt_t",
                             bufs=2)
                nc.sync.dma_start(
                    xt[:], xt_d[b].rearrange("(t p) s -> p t s", p=128)[:, :, sl]
                )
                # P1 accumulators borrow the P2 pools' banks (phases never
                # hold them concurrently): 2 from stpp + 2 from otpp.
                acc = [
                    stpp.tile([128, c.QCH], F32, name="acc0", tag="stp"),
                    stpp.tile([128, c.QCH], F32, name="acc1", tag="stp"),
                    otpp.tile([128, c.QCH], F32, name="acc2", tag="ot"),
                    otpp.tile([128, c.QCH], F32, name="acc3", tag="ot"),
                ]
                for dt in range(c.DT):
                    st, sp = dt == 0, dt == c.DT - 1
                    for h in range(c.HQC):
                        nc.tensor.matmul(
                            acc[h][:],
                            wq_sb[:, dt, h * c.HD:(h + 1) * c.HD],
                            xt[:, dt, :], start=st, stop=sp,
                        )
                    nc.tensor.matmul(
                        acc[c.HQC][:], wk_sb[:, dt, :], xt[:, dt, :],
                        start=st, stop=sp,
                    )
                    nc.tensor.matmul(
                        acc[c.HQC + 1][:], wv_sb[:, dt, :], xt[:, dt, :],
                        start=st, stop=sp,
                    )
                # evacuate PSUM -> bf16 SBUF (Act + DVE split)
                nc.scalar.copy(qts[0][:, sl], acc[0][:])
                nc.vector.tensor_copy(qts[1][:, sl], acc[1][:])
                nc.scalar.copy(kt_sb[:, sl], acc[c.HQC][:])
                nc.vector.tensor_copy(vt_sb[:, sl], acc[c.HQC + 1][:])

                # rope on this chunk of q0/q1/k (in place); the cross-partition
                # pair swap runs on the PE via a permutation matmul.
                for t in (qts[0], qts[1], kt_sb):
                    swp = mixp.tile([128, c.QCH], F32, name="swp", tag="mix")
                    nc.tensor.matmul(swp[:], pm_sb[:], t[:, sl])
                    tmp = ptp.tile([128, c.QCH], BF16, name="rtmp", tag="rtmp",
                                   bufs=2)
                    nc.vector.tensor_tensor(tmp[:], swp[:], rb_sb[:, sl], MUL)
                    nc.vector.tensor_tensor(t[:, sl], t[:, sl], ra_sb[:, sl], MUL)
                    nc.vector.tensor_tensor(t[:, sl], t[:, sl], tmp[:], ADD)

                # V^T -> V natural via PE transposes
                for i in range(c.RB):
                    st_i = sc * c.RB + i
                    tp = mixp.tile([128, 128], BF16, name="tp", tag="mix")
                    nc.tensor.transpose(
                        tp[:], vt_sb[:, st_i * 128:(st_i + 1) * 128], id_sb[:]
                    )
                    nc.scalar.copy(vn[:, st_i, :], tp[:])

            # ============ Phase 2: causal attention, S^T layout ============
            ats = [
                atp.tile([128, c.S], BF16, name=f"at{h}", tag=f"at{h}", bufs=2)
                for h in range(c.HQC)
            ]
            for h in range(c.HQC):
                qt = qts[h]
                for qc in range(c.NQC):
                    qs = qc * c.QCH
                    nkt = c.RB * (qc + 1)
                    ot = otpp.tile([128, c.QCH], F32, name="ot", tag="ot")
                    # softmax denominator accumulator: ones^T @ pt summed over
                    # the kt loop on the PE (bf16 matmul, fp32 PSUM accum)
                    zps = mixp.tile([1, c.QCH], F32, name="zps", tag="mix")
                    # kt loop software-pipelined by one iteration: the PE
                    # issues score(kt+1) before the exp(kt)-dependent colsum
                    # and PV matmuls so it never idles waiting on the Act exp.
                    def consume(pt, lv, kt):
                        nc.tensor.matmul(
                            zps[:, lv:], ones_b[:], pt[:, lv:],
                            start=(kt == 0), stop=(kt == nkt - 1),
                            skip_group_check=True,
                        )
                        nc.tensor.matmul(
                            ot[:, lv:], vn[:, kt, :], pt[:, lv:],
                            start=(kt == 0), stop=(kt == nkt - 1),
                            skip_group_check=True,
                        )

                    pending = None
                    for kt in range(nkt):
                        ridx = kt - (nkt - c.RB)
                        lv = 128 * ridx if ridx > 0 else 0  # first live column
                        stp = stpp.tile([128, c.QCH], F32, name="stp", tag="stp")
                        nc.tensor.matmul(
                            stp[:, lv:],
                            kt_sb[:, kt * 128:(kt + 1) * 128],
                            qt[:, qs + lv:qs + c.QCH],
                        )
                        if pending is not None:
                            consume(*pending)
                        pt = ptp.tile([128, c.QCH], BF16, name="pt", tag="pt",
                                      bufs=6)
                        nc.scalar.activation(
                            pt[:, lv:], stp[:, lv:], AF.Exp, scale=scale
                        )
                        if ridx >= 0:  # mask the diagonal 128-wide block
                            nc.vector.tensor_tensor(
                                pt[:, lv:lv + 128], pt[:, lv:lv + 128],
                                cm_sb[:], MUL,
                            )
                        pending = (pt, lv, kt)
                    consume(*pending)
                    # reciprocal + broadcast of the denominator row
                    zr = rsp.tile([1, c.QCH], F32, name="zr", tag="zr", bufs=2)
                    nc.vector.reciprocal_approx_fast(zr[:], zps[:])
                    zb = rsp.tile([128, c.QCH], F32, name="zb", tag="zb",
                                  bufs=2)
                    nc.gpsimd.partition_broadcast(zb[:], zr[:])
                    nc.vector.tensor_tensor(ats[h][:, qs:qs + c.QCH], ot[:],
                                            zb[:], MUL)

            # ============ Phase 3: output projection (partial of wo) ============
            for qt_i in range(c.KT):
                orow = orp.tile([128, c.D], BF16, name="orow", tag="orow", bufs=2)
                for dc in range(c.NDC):
                    dsl = slice(dc * c.QCH, (dc + 1) * c.QCH)
                    o3 = mixp.tile([128, c.QCH], F32, name="o3", tag="mix")
                    for h in range(c.HQC):
                        nc.tensor.matmul(
                            o3[:],
                            ats[h][:, qt_i * 128:(qt_i + 1) * 128],
                            wo_sb[:, h, dsl],
                            start=(h == 0), stop=(h == c.HQC - 1),
                        )
                    if dc % 2 == 0:
                        nc.scalar.copy(orow[:, dsl], o3[:])
                    else:
                        nc.vector.tensor_copy(orow[:, dsl], o3[:])
                nc.sync.dma_start(
                    out_d[b, qt_i * 128:(qt_i + 1) * 128, :], orow[:]
                )

    nc.compile()
    nc.finalize()
    return nc


# ---------------------------------------------------------------------------
# Host-side sharding / gathering
# ---------------------------------------------------------------------------

def host_prep(x, freq_cis, wq, wk, wv, wo, n_cores, cfg: Cfg):
    """Build per-core input maps (numpy only)."""
    c = cfg
    B, S, D, HD, HQC = c.B, c.S, c.D, c.HD, c.HQC
    H = wq.shape[0] // HD
    HKV = wk.shape[0] // HD
    rep = H // HKV

    x = np.asarray(x, np.float32)
    freq_cis = np.asarray(freq_cis, np.float32)
    wq = np.asarray(wq, np.float32)
    wk = np.asarray(wk, np.float32)
    wv = np.asarray(wv, np.float32)
    wo = np.asarray(wo, np.float32)

    xT = np.ascontiguousarray(x.transpose(0, 2, 1)).astype(BF)  # [B, D, S]

    # rope tables, interleaved layout: out[p] = ra[p]*t[p] + rb[p]*t[partner(p)]
    # with partner(2p) = 2p+1, partner(2p+1) = 2p
    a = freq_cis[:, :, 0, 0].T  # [HD/2, S]
    bb = freq_cis[:, :, 0, 1].T
    cc = freq_cis[:, :, 1, 0].T
    dd = freq_cis[:, :, 1, 1].T
    S_ = freq_cis.shape[0]
    ra = np.empty((HD, S_), np.float32)
    rb = np.empty((HD, S_), np.float32)
    ra[0::2], ra[1::2] = a, dd
    rb[0::2], rb[1::2] = bb, cc

    # pair-swap permutation matrix (symmetric involution)
    pm = np.zeros((HD, HD), np.float32)
    idx = np.arange(HD)
    pm[idx, idx ^ 1] = 1.0

    # causal mask for the diagonal 128x128 block: keep k <= q
    ks = np.arange(128)[:, None]
    qs = np.arange(128)[None, :]
    cm = (ks <= qs).astype(np.float32)
    ident = np.eye(128, dtype=np.float32)

    in_maps = []
    for core in range(n_cores):
        h0 = core * HQC
        kvh = h0 // rep
        wq_c = wq[h0 * HD:(h0 + HQC) * HD]
        wk_c = wk[kvh * HD:(kvh + 1) * HD]
        wv_c = wv[kvh * HD:(kvh + 1) * HD]
        wo_c = wo[:, h0 * HD:(h0 + HQC) * HD]
        in_maps.append({
            "xt": xT,
            "wqt": np.ascontiguousarray(wq_c.T).astype(BF),
            "wkt": np.ascontiguousarray(wk_c.T).astype(BF),
            "wvt": np.ascontiguousarray(wv_c.T).astype(BF),
            "wot": np.ascontiguousarray(wo_c.T).astype(BF),
            "ra": ra.astype(BF),
            "rb": rb.astype(BF),
            "cm": cm.astype(BF),
            "id": ident.astype(BF),
            "pm": pm.astype(BF),
        })
    return in_maps


def run(inputs: dict, n_cores: int = 8, cfg: Cfg = Cfg(), trace: bool = False):
    in_maps = host_prep(
        inputs["x"], inputs["freq_cis"], inputs["wq"], inputs["wk"],
        inputs["wv"], inputs["wo"], n_cores, cfg,
    )
    nc = build_program(cfg)
    res = run_bass_kernel_spmd(nc, in_maps, list(range(n_cores)), trace=trace)
    out = res.results[0]["partial"].astype(np.float32)
    for core in range(1, n_cores):
        out += res.results[core]["partial"].astype(np.float32)
    return out, res


def kernel(**inputs) -> np.ndarray:
    out, _ = run(inputs, n_cores=8, cfg=Cfg())
    return out


# revision 27
# speedup vs baseline: 1.1786x; 1.0082x over previous
"""Trainium2 Bass kernel for GQA attention (RoPE + causal) with output projection.

Strategy: tensor-parallel over heads across 8 NeuronCores. Core c computes
q-heads {2c, 2c+1} and kv-head c//2, projects with its weight slices, runs
causal flash-style attention in scores-transposed layout, applies its slice
of wo, and returns a full-shape partial output (bf16). The host sums the 8
partials (the all-reduce of the TP layout).

v2 changes vs baseline:
- all matmul operands bf16 (host-converted): enables Fast Weight Load,
  halves DMA + SBUF traffic; PSUM accumulation stays fp32.
- causal band tiles compute only live columns (skips ~15% of P2 work).
- softmax denominator: reciprocal_approx_fast (5x faster than reciprocal).
- rsum accumulation moved to the (otherwise idle) GpSimd engine where it is
  SBUF-only; PSUM evacuations split between Act and DVE.
- 8-bank PSUM budget: shared 4-slot rotating pool (P1 accs / rope swaps /
  transposes / P3 tiles / softmax sums) + 2 score bufs + 2 PV accumulators.
- one DMA per x chunk; bf16 partial outputs.
"""

import math
from contextlib import ExitStack
from dataclasses import dataclass

import numpy as np
import ml_dtypes

import concourse.bass as bass
import concourse.tile as tile
from concourse import bacc, mybir
from concourse.bass_utils import run_bass_kernel_spmd

F32 = mybir.dt.float32
F32R = mybir.dt.float32r
BF16 = mybir.dt.bfloat16
AF = mybir.ActivationFunctionType
MUL = mybir.AluOpType.mult
ADD = mybir.AluOpType.add

BF = ml_dtypes.bfloat16


@dataclass(frozen=True)
class Cfg:
    B: int = 4          # batch
    S: int = 2048       # sequence length
    D: int = 2048       # model dim
    HQC: int = 2        # q-heads per core
    HD: int = 128       # head dim (must be 128)
    QCH: int = 512      # q-chunk (matmul moving free dim)

    @property
    def DT(self):
        return self.D // 128   # d-tiles

    @property
    def KT(self):
        return self.S // 128   # k-tiles / s-tiles / q-tiles

    @property
    def NQC(self):
        return self.S // self.QCH  # q-chunks

    @property
    def RB(self):
        return self.QCH // 128     # band tiles per q-chunk

    @property
    def NDC(self):
        return self.D // self.QCH  # dout chunks


def r(ap):
    """View an fp32 AP as float32r for full-rate PE matmuls."""
    return ap.bitcast(F32R)


def build_program(cfg: Cfg):
    """Build + compile the single-core Bass program (same program on every core)."""
    c = cfg
    assert c.HD == 128
    nc = bacc.Bacc("TRN2", target_bir_lowering=False, debug=False)

    xt_d = nc.dram_tensor("xt", [c.B, c.D, c.S], BF16, kind="ExternalInput")
    wqt_d = nc.dram_tensor("wqt", [c.D, c.HQC * c.HD], BF16, kind="ExternalInput")
    wkt_d = nc.dram_tensor("wkt", [c.D, c.HD], BF16, kind="ExternalInput")
    wvt_d = nc.dram_tensor("wvt", [c.D, c.HD], BF16, kind="ExternalInput")
    wot_d = nc.dram_tensor("wot", [c.HQC * c.HD, c.D], BF16, kind="ExternalInput")
    ra_d = nc.dram_tensor("ra", [c.HD, c.S], BF16, kind="ExternalInput")
    rb_d = nc.dram_tensor("rb", [c.HD, c.S], BF16, kind="ExternalInput")
    cm_d = nc.dram_tensor("cm", [128, 128], BF16, kind="ExternalInput")
    id_d = nc.dram_tensor("id", [128, 128], BF16, kind="ExternalInput")
    pm_d = nc.dram_tensor("pm", [128, 128], BF16, kind="ExternalInput")
    out_d = nc.dram_tensor("partial", [c.B, c.S, c.D], BF16, kind="ExternalOutput")

    scale = 1.0 / math.sqrt(c.HD)

    with tile.TileContext(nc) as tc, ExitStack() as ctx:
        const = ctx.enter_context(tc.tile_pool(name="const", bufs=1))
        xp = ctx.enter_context(tc.tile_pool(name="xp", bufs=1))
        qk = ctx.enter_context(tc.tile_pool(name="qk", bufs=1))
        ptp = ctx.enter_context(tc.tile_pool(name="ptp", bufs=1))
        rsp = ctx.enter_context(tc.tile_pool(name="rsp", bufs=1))
        atp = ctx.enter_context(tc.tile_pool(name="atp", bufs=1))
        orp = ctx.enter_context(tc.tile_pool(name="orp", bufs=1))
        # PSUM: mix (3) + stp (3, skew-2 score pipeline) + otp (2) = 8 banks
        mixp = ctx.enter_context(
            tc.tile_pool(name="mixp", bufs=3, space=bass.MemorySpace.PSUM)
        )
        stpp = ctx.enter_context(
            tc.tile_pool(name="stpp", bufs=3, space=bass.MemorySpace.PSUM)
        )
        otpp = ctx.enter_context(
            tc.tile_pool(name="otpp", bufs=2, space=bass.MemorySpace.PSUM)
        )

        # ---- resident constants ----
        wq_sb = const.tile([128, c.DT, c.HQC * c.HD], BF16, name="wq_sb")
        nc.sync.dma_start(wq_sb[:], wqt_d.rearrange("(t p) h -> p t h", p=128))
        wk_sb = const.tile([128, c.DT, c.HD], BF16, name="wk_sb")
        nc.sync.dma_start(wk_sb[:], wkt_d.rearrange("(t p) h -> p t h", p=128))
        wv_sb = const.tile([128, c.DT, c.HD], BF16, name="wv_sb")
        nc.sync.dma_start(wv_sb[:], wvt_d.rearrange("(t p) h -> p t h", p=128))
        wo_sb = const.tile([128, c.HQC, c.D], BF16, name="wo_sb")
        nc.sync.dma_start(wo_sb[:], wot_d.rearrange("(h p) d -> p h d", p=128))
        ra_sb = const.tile([128, c.S], BF16, name="ra_sb")
        nc.sync.dma_start(ra_sb[:], ra_d[:])
        rb_sb = const.tile([128, c.S], BF16, name="rb_sb")
        nc.sync.dma_start(rb_sb[:], rb_d[:])
        cm_sb = const.tile([128, 128], BF16, name="cm_sb")
        nc.sync.dma_start(cm_sb[:], cm_d[:])
        id_sb = const.tile([128, 128], BF16, name="id_sb")
        nc.sync.dma_start(id_sb[:], id_d[:])
        pm_sb = const.tile([128, 128], BF16, name="pm_sb")
        nc.sync.dma_start(pm_sb[:], pm_d[:])
        ones_b = const.tile([128, 1], BF16, name="ones_b")
        nc.vector.memset(ones_b[:], 1.0)

        for b in range(c.B):
            # ============ Phase 1: Q^T / K^T / V^T projections ============
            qts = [
                qk.tile([128, c.S], BF16, name=f"qt{h}", tag=f"qt{h}", bufs=2)
                for h in range(c.HQC)
            ]
            kt_sb = qk.tile([128, c.S], BF16, name="kt_sb", tag="kt_sb", bufs=2)
            vt_sb = qk.tile([128, c.S], BF16, name="vt_sb", tag="vt_sb", bufs=2)
            vn = qk.tile([128, c.KT, c.HD], BF16, name="vn", tag="vn", bufs=2)

            for sc in range(c.NQC):
                sl = slice(sc * c.QCH, (sc + 1) * c.QCH)
                xt = xp.tile([128, c.DT, c.QCH], BF16, name="xt_t", tag="xt_t",
                             bufs=2)
                nc.sync.dma_start(
                    xt[:], xt_d[b].rearrange("(t p) s -> p t s", p=128)[:, :, sl]
                )
                # P1 accumulators borrow the P2 pools' banks (phases never
                # hold them concurrently): 2 from stpp + 2 from otpp.
                acc = [
                    stpp.tile([128, c.QCH], F32, name="acc0", tag="stp"),
                    stpp.tile([128, c.QCH], F32, name="acc1", tag="stp"),
                    otpp.tile([128, c.QCH], F32, name="acc2", tag="ot"),
                    otpp.tile([128, c.QCH], F32, name="acc3", tag="ot"),
                ]
                for dt in range(c.DT):
                    st, sp = dt == 0, dt == c.DT - 1
                    for h in range(c.HQC):
                        nc.tensor.matmul(
                            acc[h][:],
                            wq_sb[:, dt, h * c.HD:(h + 1) * c.HD],
                            xt[:, dt, :], start=st, stop=sp,
                        )
                    nc.tensor.matmul(
                        acc[c.HQC][:], wk_sb[:, dt, :], xt[:, dt, :],
                        start=st, stop=sp,
                    )
                    nc.tensor.matmul(
                        acc[c.HQC + 1][:], wv_sb[:, dt, :], xt[:, dt, :],
                        start=st, stop=sp,
                    )
                # evacuate PSUM -> bf16 SBUF (Act + DVE split)
                nc.scalar.copy(qts[0][:, sl], acc[0][:])
                nc.vector.tensor_copy(qts[1][:, sl], acc[1][:])
                nc.scalar.copy(kt_sb[:, sl], acc[c.HQC][:])
                nc.vector.tensor_copy(vt_sb[:, sl], acc[c.HQC + 1][:])

                # rope on this chunk of q0/q1/k (in place); the cross-partition
                # pair swap runs on the PE via a permutation matmul.
                for t in (qts[0], qts[1], kt_sb):
                    swp = mixp.tile([128, c.QCH], F32, name="swp", tag="mix")
                    nc.tensor.matmul(swp[:], pm_sb[:], t[:, sl])
                    tmp = ptp.tile([128, c.QCH], BF16, name="rtmp", tag="rtmp",
                                   bufs=2)
                    nc.vector.tensor_tensor(tmp[:], swp[:], rb_sb[:, sl], MUL)
                    nc.vector.tensor_tensor(t[:, sl], t[:, sl], ra_sb[:, sl], MUL)
                    nc.vector.tensor_tensor(t[:, sl], t[:, sl], tmp[:], ADD)

                # V^T -> V natural via PE transposes
                for i in range(c.RB):
                    st_i = sc * c.RB + i
                    tp = mixp.tile([128, 128], BF16, name="tp", tag="mix")
                    nc.tensor.transpose(
                        tp[:], vt_sb[:, st_i * 128:(st_i + 1) * 128], id_sb[:]
                    )
                    nc.scalar.copy(vn[:, st_i, :], tp[:])

            # ============ Phase 2: causal attention, S^T layout ============
            ats = [
                atp.tile([128, c.S], BF16, name=f"at{h}", tag=f"at{h}", bufs=2)
                for h in range(c.HQC)
            ]
            for h in range(c.HQC):
                qt = qts[h]
                for qc in range(c.NQC):
                    qs = qc * c.QCH
                    nkt = c.RB * (qc + 1)
                    ot = otpp.tile([128, c.QCH], F32, name="ot", tag="ot")
                    # softmax denominator accumulator: ones^T @ pt summed over
                    # the kt loop on the PE (bf16 matmul, fp32 PSUM accum)
                    zps = mixp.tile([1, c.QCH], F32, name="zps", tag="mix")
                    # kt loop software-pipelined two iterations deep: the PE
                    # issues score(kt+1)/score(kt+2) before the exp(kt)-gated
                    # colsum and PV matmuls so it never idles on the Act exp.
                    def consume(pt, lv, kt):
                        nc.tensor.matmul(
                            zps[:, lv:], ones_b[:], pt[:, lv:],
                            start=(kt == 0), stop=(kt == nkt - 1),
                            skip_group_check=True,
                        )
                        nc.tensor.matmul(
                            ot[:, lv:], vn[:, kt, :], pt[:, lv:],
                            start=(kt == 0), stop=(kt == nkt - 1),
                            skip_group_check=True,
                        )

                    pending = []
                    for kt in range(nkt):
                        ridx = kt - (nkt - c.RB)
                        lv = 128 * ridx if ridx > 0 else 0  # first live column
                        stp = stpp.tile([128, c.QCH], F32, name="stp", tag="stp")
                        nc.tensor.matmul(
                            stp[:, lv:],
                            kt_sb[:, kt * 128:(kt + 1) * 128],
                            qt[:, qs + lv:qs + c.QCH],
                        )
                        if len(pending) >= 2:
                            consume(*pending.pop(0))
                        pt = ptp.tile([128, c.QCH], BF16, name="pt", tag="pt",
                                      bufs=6)
                        nc.scalar.activation(
                            pt[:, lv:], stp[:, lv:], AF.Exp, scale=scale
                        )
                        if ridx >= 0:  # mask the diagonal 128-wide block
                            nc.vector.tensor_tensor(
                                pt[:, lv:lv + 128], pt[:, lv:lv + 128],
                                cm_sb[:], MUL,
                            )
                        pending.append((pt, lv, kt))
                    for p in pending:
                        consume(*p)
                    # reciprocal + broadcast of the denominator row
                    zr = rsp.tile([1, c.QCH], F32, name="zr", tag="zr", bufs=2)
                    nc.vector.reciprocal_approx_fast(zr[:], zps[:])
                    zb = rsp.tile([128, c.QCH], F32, name="zb", tag="zb",
                                  bufs=2)
                    nc.gpsimd.partition_broadcast(zb[:], zr[:])
                    nc.vector.tensor_tensor(ats[h][:, qs:qs + c.QCH], ot[:],
                                            zb[:], MUL)

            # ============ Phase 3: output projection (partial of wo) ============
            for qt_i in range(c.KT):
                orow = orp.tile([128, c.D], BF16, name="orow", tag="orow", bufs=2)
                for dc in range(c.NDC):
                    dsl = slice(dc * c.QCH, (dc + 1) * c.QCH)
                    o3 = mixp.tile([128, c.QCH], F32, name="o3", tag="mix")
                    for h in range(c.HQC):
                        nc.tensor.matmul(
                            o3[:],
                            ats[h][:, qt_i * 128:(qt_i + 1) * 128],
                            wo_sb[:, h, dsl],
                            start=(h == 0), stop=(h == c.HQC - 1),
                        )
                    if dc % 2 == 0:
                        nc.scalar.copy(orow[:, dsl], o3[:])
                    else:
                        nc.vector.tensor_copy(orow[:, dsl], o3[:])
                nc.sync.dma_start(
                    out_d[b, qt_i * 128:(qt_i + 1) * 128, :], orow[:]
                )

    nc.compile()
    nc.finalize()
    return nc


# ---------------------------------------------------------------------------
# Host-side sharding / gathering
# ---------------------------------------------------------------------------

def host_prep(x, freq_cis, wq, wk, wv, wo, n_cores, cfg: Cfg):
    """Build per-core input maps (numpy only)."""
    c = cfg
    B, S, D, HD, HQC = c.B, c.S, c.D, c.HD, c.HQC
    H = wq.shape[0] // HD
    HKV = wk.shape[0] // HD
    rep = H // HKV

    x = np.asarray(x, np.float32)
    freq_cis = np.asarray(freq_cis, np.float32)
    wq = np.asarray(wq, np.float32)
    wk = np.asarray(wk, np.float32)
    wv = np.asarray(wv, np.float32)
    wo = np.asarray(wo, np.float32)

    xT = np.ascontiguousarray(x.transpose(0, 2, 1)).astype(BF)  # [B, D, S]

    # rope tables, interleaved layout: out[p] = ra[p]*t[p] + rb[p]*t[partner(p)]
    # with partner(2p) = 2p+1, partner(2p+1) = 2p
    a = freq_cis[:, :, 0, 0].T  # [HD/2, S]
    bb = freq_cis[:, :, 0, 1].T
    cc = freq_cis[:, :, 1, 0].T
    dd = freq_cis[:, :, 1, 1].T
    S_ = freq_cis.shape[0]
    ra = np.empty((HD, S_), np.float32)
    rb = np.empty((HD, S_), np.float32)
    ra[0::2], ra[1::2] = a, dd
    rb[0::2], rb[1::2] = bb, cc

    # pair-swap permutation matrix (symmetric involution)
    pm = np.zeros((HD, HD), np.float32)
    idx = np.arange(HD)
    pm[idx, idx ^ 1] = 1.0

    # causal mask for the diagonal 128x128 block: keep k <= q
    ks = np.arange(128)[:, None]
    qs = np.arange(128)[None, :]
    cm = (ks <= qs).astype(np.float32)
    ident = np.eye(128, dtype=np.float32)

    in_maps = []
    for core in range(n_cores):
        h0 = core * HQC
        kvh = h0 // rep
        wq_c = wq[h0 * HD:(h0 + HQC) * HD]
        wk_c = wk[kvh * HD:(kvh + 1) * HD]
        wv_c = wv[kvh * HD:(kvh + 1) * HD]
        wo_c = wo[:, h0 * HD:(h0 + HQC) * HD]
        in_maps.append({
            "xt": xT,
            "wqt": np.ascontiguousarray(wq_c.T).astype(BF),
            "wkt": np.ascontiguousarray(wk_c.T).astype(BF),
            "wvt": np.ascontiguousarray(wv_c.T).astype(BF),
            "wot": np.ascontiguousarray(wo_c.T).astype(BF),
            "ra": ra.astype(BF),
            "rb": rb.astype(BF),
            "cm": cm.astype(BF),
            "id": ident.astype(BF),
            "pm": pm.astype(BF),
        })
    return in_maps


def run(inputs: dict, n_cores: int = 8, cfg: Cfg = Cfg(), trace: bool = False):
    in_maps = host_prep(
        inputs["x"], inputs["freq_cis"], inputs["wq"], inputs["wk"],
        inputs["wv"], inputs["wo"], n_cores, cfg,
    )
    nc = build_program(cfg)
    res = run_bass_kernel_spmd(nc, in_maps, list(range(n_cores)), trace=trace)
    out = res.results[0]["partial"].astype(np.float32)
    for core in range(1, n_cores):
        out += res.results[core]["partial"].astype(np.float32)
    return out, res


def kernel(**inputs) -> np.ndarray:
    out, _ = run(inputs, n_cores=8, cfg=Cfg())
    return out
